# revision 37
# baseline (speedup 1.0000x reference)
"""GQA attention block (B=1, S=2048, D=2048, H=32, G=8, HD=64) on 8 trn2 cores.

Sharding: tensor-parallel over heads/KV-groups. Core c owns q-heads
4c..4c+3 and KV group c. Wq/Wk/Wv column-parallel, Wo row-parallel;
each core computes a partial [S, D] output, host sums the 8 partials.

Per-core dataflow (all matmuls bf16, stats f32), software-pipelined so
PE / ACT / DVE overlap and the PE rarely idles long enough for the HAM
clock gate to re-throttle it:
  Phase A: half the gate proj ([e,s] layout), i-outer accumulation
    paced by the streaming xt/wg DMAs; sigmoid via tanh. The other
    half runs as PE filler inside phase B.
  Phase B: qkv proj -> psum [s,384] per s-tile j, in groups of 4 with
    the RMS stats batched into one [128,20] Newton chain; RoPE on DVE
    (bf16, 2x mode); psum->sbuf staging copies ride the ACT engine.
    PE-transposes of group g-1 are interleaved after the MMs of group
    g so the PE never waits on the DVE chain.
  Phase C: attention, q-slice outer / head-PAIR inner. The two heads
    of a pair are computed by two concurrent row-tiled score MMs
    (K=64 each, partition halves of kT/qT duplicated/packed) into the
    two banks of one [128,1024] psum pair tile; ONE exp covers both
    heads via a strided 2-region view. The causal boundary block is
    tri-multiplied into a side tile so the mask-free remainder feeds
    the ctx MMs without a DVE hop. den goes through a lane-spread
    reciprocal ([1,512] DMA-reshaped to [128,4]), is PE-broadcast one
    pair late, and out-proj chunks are interleaved as PE filler.
    Partials written bf16, host sums the 8 cores.
"""

import numpy as np
import ml_dtypes

import concourse.bass as bass
import concourse.tile as tile
from concourse import bacc, mybir
from concourse.bass_utils import run_bass_kernel_spmd
from concourse.masks import make_identity

BF16 = mybir.dt.bfloat16
F32 = mybir.dt.float32
NBF = ml_dtypes.bfloat16

S = 2048
D = 2048
H = 32
G = 8
HD = 64
NCORE = 8
NHL = H // NCORE          # 4 q heads per core
EL = NHL * HD             # 256 local q (and gate, and ctx) features
QK = EL + HD              # 320: q + k features
QKV = QK + HD             # 384: q + k + v
P = 128
NS = S // P               # 16 s-tiles
ND = D // P               # 16 d-tiles
SQ = 512
NSQ = S // SQ             # 4 sq slices
NB = QK // HD             # 5 (hd,) blocks in the q|k strip
SCALE = HD ** -0.5
EPS = 1e-6


def _v(ap, dims, extra_offset=0):
    """Reshape the free dims of a 2D AP into `dims` ([step, count] pairs),
    keeping the partition dim."""
    return bass.AP(
        tensor=ap.tensor,
        offset=ap.offset + extra_offset,
        ap=[list(ap.ap[0])] + [list(d) for d in dims],
    )


def _mk(pool, shape, dtype, tag, bufs=None):
    return pool.tile(shape, dtype, tag=tag, name=tag, bufs=bufs)


def build_nc():
    nc = bacc.Bacc("TRN2", target_bir_lowering=False, debug=False,
                   num_devices=NCORE)

    xt = nc.dram_tensor("xt", [D, S], BF16, kind="ExternalInput").ap()
    wqkv = nc.dram_tensor("wqkv", [D, QKV], BF16, kind="ExternalInput").ap()
    wg = nc.dram_tensor("wg", [D, EL], BF16, kind="ExternalInput").ap()
    wo = nc.dram_tensor("wo", [EL, D], BF16, kind="ExternalInput").ap()
    cos5 = nc.dram_tensor("cos5", [S, QK], BF16, kind="ExternalInput").ap()
    sin5 = nc.dram_tensor("sin5", [S, QK], BF16, kind="ExternalInput").ap()
    tri = nc.dram_tensor("tri", [P, P], BF16, kind="ExternalInput").ap()
    out = nc.dram_tensor("out", [S, D], BF16, kind="ExternalOutput").ap()

    with tile.TileContext(nc) as tc:
        with (
            tc.tile_pool(name="persist", bufs=1) as pp,
            tc.tile_pool(name="work", bufs=2) as wp,
            tc.tile_pool(name="stats", bufs=3) as sp,
            tc.tile_pool(name="probs", bufs=4) as prp,
            tc.tile_pool(name="outc", bufs=4) as ocp,
            tc.tile_pool(name="psum", bufs=4, space="PSUM") as psp,
        ):
            # ---- persistent loads: wg+xt first (phase A is paced by
            # these), then wqkv (phase B), cos/sin, tri, wo ----
            qeng = [nc.sync, nc.scalar, nc.sync, nc.scalar]
            xts, wgs = [], []
            for i in range(ND):
                tg = _mk(pp, [P, EL], BF16, f"wg{i}")
                qeng[i % 4].dma_start(out=tg, in_=wg[i * P:(i + 1) * P, :])
                wgs.append(tg)
                t = _mk(pp, [P, S], BF16, f"xt{i}")
                qeng[(i + 1) % 4].dma_start(out=t,
                                            in_=xt[i * P:(i + 1) * P, :])
                xts.append(t)
            wqkvs = []
            for i in range(ND):
                t = _mk(pp, [P, QKV], BF16, f"wqkv{i}")
                qeng[i % 4].dma_start(out=t, in_=wqkv[i * P:(i + 1) * P, :])
                wqkvs.append(t)
            coss, sins = [], []
            for j in range(NS):
                tc_ = _mk(pp, [P, QK], BF16, f"cos{j}")
                nc.sync.dma_start(out=tc_, in_=cos5[j * P:(j + 1) * P, :])
                coss.append(tc_)
                ts_ = _mk(pp, [P, QK], BF16, f"sin{j}")
                nc.sync.dma_start(out=ts_, in_=sin5[j * P:(j + 1) * P, :])
                sins.append(ts_)
            tri_sb = _mk(pp, [P, P], BF16, "tri")
            nc.sync.dma_start(out=tri_sb, in_=tri)
            wos = []
            for e in range(2):
                t = _mk(pp, [P, D], BF16, f"wo{e}")
                nc.sync.dma_start(out=t, in_=wo[e * P:(e + 1) * P, :])
                wos.append(t)
            ident = _mk(pp, [P, P], BF16, "ident")
            make_identity(nc, ident)
            ones_b = _mk(pp, [1, P], BF16, "ones_b")
            nc.vector.memset(ones_b, 1.0)

            # persistent intermediate tensors
            # qpair[p][q]: rows 0-63 = qT of head 2p, 64-127 = head 2p+1
            # kt2[q]: kT duplicated on both partition halves (for the
            # row-tiled concurrent score matmuls)
            qpair = [[_mk(pp, [P, SQ], BF16, f"qp{p}_{q}")
                      for q in range(NSQ)] for p in range(2)]
            kt2 = [_mk(pp, [P, SQ], BF16, f"kt{q}") for q in range(NSQ)]
            vs = [_mk(pp, [P, HD + 1], BF16, f"v{j}") for j in range(NS)]
            for j in range(NS):
                nc.vector.memset(vs[j][:, HD:HD + 1], 1.0)
            gus2 = [[_mk(pp, [P, 2 * SQ], BF16, f"gu{p}_{qq}")
                     for qq in range(2)] for p in range(2)]
            ctxgs = [[_mk(pp, [P, SQ], BF16, f"cg{p}_{q}") for q in range(NSQ)]
                     for p in range(2)]

            # ---- phase A: gate projection for p=0 only, i-outer (paced
            # by the streaming wg/xt DMAs). The p=1 half runs as PE
            # filler inside the DVE-bound phase B (one dense 16-MM block
            # per stats group).
            ss_gs = [_mk(psp, [P, 2 * SQ], F32, "ss", bufs=2)
                     for _ in range(2)]
            for i in range(ND):
                for q in range(NSQ):
                    nc.tensor.matmul(
                        ss_gs[q // 2][:, (q % 2) * SQ:(q % 2 + 1) * SQ],
                        wgs[i][:, 0:P], xts[i][:, q * SQ:(q + 1) * SQ],
                        start=(i == 0), stop=(i == ND - 1))
            # u = 1 + tanh(g/2) = 2*sigmoid(g)
            for qq in range(2):
                nc.scalar.activation(gus2[0][qq], ss_gs[qq],
                                     mybir.ActivationFunctionType.Tanh,
                                     scale=0.5)
                nc.vector.tensor_scalar_add(gus2[0][qq], gus2[0][qq], 1.0)

            def emit_gate_block(q):
                ps_gq = _mk(psp, [P, SQ], F32, "ps")
                for i in range(ND):
                    nc.tensor.matmul(
                        ps_gq, wgs[i][:, P:2 * P],
                        xts[i][:, q * SQ:(q + 1) * SQ],
                        start=(i == 0), stop=(i == ND - 1))
                dst = gus2[1][q // 2][:, (q % 2) * SQ:(q % 2 + 1) * SQ]
                nc.scalar.activation(dst, ps_gq,
                                     mybir.ActivationFunctionType.Tanh,
                                     scale=0.5)
                nc.vector.tensor_scalar_add(dst, dst, 1.0)

            # ---- phase B: qkv projection + norm + rope + transpose ----
            # Processed in groups of 4 s-tiles: the RMS stats of a group
            # are batched into one [128, 20] Newton chain (tiny DVE ops
            # amortize 4x). PE transposes for group g-1 are emitted after
            # the MMs of group g, so the PE never waits on the DVE chain.
            pend_tr = None   # (j, qkr tile)

            def emit_transposes(j, qkr):
                jq, jc = j // 4, (j % 4) * P
                # all three transposes share one 2-bank "ss" tile (bf16,
                # all regions within bank 0's byte range is fine: each
                # matmul output stays inside one bank)
                tt = _mk(psp, [P, 4 * SQ], BF16, "ss", bufs=2)
                for p in range(2):
                    nc.tensor.transpose(tt[:, p * P:(p + 1) * P],
                                        qkr[:, p * P:(p + 1) * P], ident)
                    nc.scalar.copy(qpair[p][jq][:, jc:jc + P],
                                   tt[:, p * P:(p + 1) * P])
                nc.tensor.transpose(tt[:HD, 2 * P:3 * P],
                                    qkr[:, 2 * P:2 * P + HD], ident)
                nc.scalar.copy(kt2[jq][:HD, jc:jc + P],
                               tt[:HD, 2 * P:3 * P])
                # duplicate k onto the upper partition half via SBUF->SBUF
                # DMA (partition-crossing moves are DMA-only)
                nc.sync.dma_start(out=kt2[jq][HD:, jc:jc + P],
                                  in_=kt2[jq][:HD, jc:jc + P])

            GJ = 4                       # s-tiles per stats group
            pend_trs = []                # [(j, qkr), ...] of group g-1
            for g in range(NS // GJ):
                js = list(range(GJ * g, GJ * (g + 1)))
                qksbs = []
                ssg = _mk(sp, [P, NB * GJ], F32, "ssg")
                for gi, j in enumerate(js):
                    ps_qkv = _mk(psp, [P, QKV], F32, "ps")
                    for i in range(ND):
                        nc.tensor.matmul(
                            ps_qkv, xts[i][:, j * P:(j + 1) * P], wqkvs[i],
                            start=(i == 0), stop=(i == ND - 1))
                    if pend_trs:
                        emit_transposes(*pend_trs.pop(0))
                    if gi == 1:
                        emit_gate_block(g)   # PE filler under the DVE chain
                    # v straight to SBUF (ones column preset in prologue);
                    # psum->sbuf copies ride the otherwise-idle ACT engine
                    nc.scalar.copy(vs[j][:, :HD], ps_qkv[:, QK:QKV])
                    # park q|k in SBUF (bf16) and batch the RMS stats
                    qksb = _mk(wp, [P, QK], BF16, "qksb", bufs=2 * GJ)
                    nc.scalar.copy(qksb, ps_qkv[:, :QK])
                    qksbs.append(qksb)
                    sqr = _mk(wp, [P, QK], F32, "sqr")
                    nc.scalar.activation(sqr, qksb,
                                         mybir.ActivationFunctionType.Square)
                    nc.vector.tensor_reduce(
                        ssg[:, NB * gi:NB * (gi + 1)],
                        _v(sqr, [[HD, NB], [1, HD]]),
                        axis=mybir.AxisListType.X, op=mybir.AluOpType.add)
                # m = mean + eps;  r = rsqrt(m): poly seed + 2 Newton steps
                NBG = NB * GJ
                m = _mk(sp, [P, NBG], F32, "m")
                nc.vector.tensor_scalar(m, ssg, 1.0 / HD, EPS,
                                        mybir.AluOpType.mult,
                                        mybir.AluOpType.add)
                mc = _mk(sp, [P, NBG], F32, "mc")
                nc.vector.tensor_scalar(mc, m, 5.5, 0.45,
                                        mybir.AluOpType.min,
                                        mybir.AluOpType.max)
                m2 = _mk(sp, [P, NBG], F32, "m2")
                nc.vector.tensor_mul(m2, mc, mc)
                lin = _mk(sp, [P, NBG], F32, "lin")
                nc.vector.tensor_scalar(lin, mc, -0.48330447, 1.51774376,
                                        mybir.AluOpType.mult,
                                        mybir.AluOpType.add)
                y = _mk(sp, [P, NBG], F32, "y")
                nc.vector.scalar_tensor_tensor(y, m2, 0.0534932, lin,
                                               mybir.AluOpType.mult,
                                               mybir.AluOpType.add)
                ytmp = _mk(sp, [P, NBG], F32, "ytmp")
                for _ in range(2):
                    nc.vector.tensor_mul(ytmp, y, y)              # y^2
                    nc.vector.tensor_mul(ytmp, ytmp, m)           # m y^2
                    nc.vector.tensor_scalar(ytmp, ytmp, -0.5, 1.5,
                                            mybir.AluOpType.mult,
                                            mybir.AluOpType.add)
                    nc.vector.tensor_mul(y, y, ytmp)
                for gi, j in enumerate(js):
                    # qkn = qk * r (broadcast r over hd)
                    qkn = _mk(wp, [P, QK], BF16, "qkn")
                    nc.vector.tensor_mul(
                        _v(qkn, [[HD, NB], [1, HD]]),
                        _v(qksbs[gi], [[HD, NB], [1, HD]]),
                        _v(y[:, NB * gi:], [[1, NB], [0, HD]]))
                    # rope: out = qkn*cos5 + rot(qkn)*sin5  (sin pre-negated
                    # on the first half on host; cos/sin include 1+norm_w)
                    t1 = _mk(wp, [P, QK], BF16, "t1")
                    nc.vector.tensor_mul(t1, qkn, coss[j])
                    t2 = _mk(wp, [P, QK], BF16, "t2")
                    rot = _v(qkn[:, :], [[HD, NB], [-32, 2], [1, 32]],
                             extra_offset=32)
                    nc.vector.tensor_mul(
                        _v(t2, [[HD, NB], [32, 2], [1, 32]]), rot,
                        _v(sins[j][:, :], [[HD, NB], [32, 2], [1, 32]]))
                    qkr = _mk(wp, [P, QK], BF16, "qkr", bufs=2 * GJ)
                    nc.vector.tensor_add(qkr, t1, t2)
                    pend_trs.append((j, qkr))
            for tr in pend_trs:
                emit_transposes(*tr)

            # ---- phase C: attention, q-slice outer / head inner, with
            # delayed den processing and interleaved out-projection ----
            def stash_ctx(h, q, ps_ctx):
                """Emitted right after the jk loop: start the lane-spread
                reciprocal (den [1,512] is DMA-reshaped to [128,4] so the
                iterative-divide runs 128 lanes wide instead of 1), then
                DMA-broadcast 0.5/den (bf16) across 64 partitions."""
                dsb = _mk(sp, [1, SQ], F32, "dsb", bufs=3)
                nc.scalar.copy(dsb, ps_ctx[HD:HD + 1, :])
                dstk = _mk(sp, [P, SQ // P], F32, "dstk", bufs=3)
                nc.sync.dma_start(out=dstk, in_=dsb)
                rsm = _mk(sp, [P, SQ // P], F32, "rsm", bufs=3)
                nc.vector.reciprocal(rsm, dstk)
                rsmb = _mk(sp, [P, SQ // P], BF16, "rsmb", bufs=3)
                nc.vector.tensor_scalar_mul(rsmb, rsm, 0.5)
                denr = _mk(sp, [1, SQ], BF16, "denrb", bufs=3)
                nc.sync.dma_start(out=denr, in_=rsmb)
                return (h, q, ps_ctx, denr)

            def emit_den(h, q, ps_ctx, denr):
                """Delayed one head-pair: PE broadcast of 0.5/den + the
                gate/den multiplies."""
                hp, ho = h // 2, (h % 2) * HD
                ps_db = _mk(psp, [P, SQ], F32, "ps")
                nc.tensor.matmul(ps_db, ones_b, denr, start=True, stop=True)
                # ctxg = ctx * u * (0.5/den)
                tmpu = _mk(wp, [HD, SQ], BF16, "tmpu")
                nc.vector.tensor_mul(
                    tmpu, ps_ctx[:HD, :],
                    gus2[hp][q // 2][ho:ho + HD,
                                     (q % 2) * SQ:(q % 2) * SQ + SQ])
                nc.vector.tensor_mul(
                    ctxgs[hp][q][ho:ho + HD, :], tmpu, ps_db[:HD, :])

            def op_chunk(jq, jj, n2):
                j = 4 * jq + jj
                jc = jj * P
                ps_o = _mk(psp, [P, 2 * SQ], F32, "ss", bufs=2)
                for half in range(2):
                    n = 2 * n2 + half
                    for e in range(2):
                        nc.tensor.matmul(
                            ps_o[:, half * SQ:(half + 1) * SQ],
                            ctxgs[e][jq][:, jc:jc + P],
                            wos[e][:, n * SQ:(n + 1) * SQ],
                            start=(e == 0), stop=(e == 1))
                oc = _mk(ocp, [P, 2 * SQ], BF16, "oc")
                nc.vector.tensor_copy(out=oc, in_=ps_o)
                nc.sync.dma_start(
                    out=out[j * P:(j + 1) * P,
                            n2 * 2 * SQ:(n2 + 1) * 2 * SQ],
                    in_=oc)

            pend_den = []      # stashes of the previous head-pair
            pend_chunks = []   # deferred out-proj (jq, jj, n) chunks
            for q in range(NSQ):
                jq = q - 1
                if jq >= 0:
                    pend_chunks += [(jq, jj, n2) for jj in range(4)
                                    for n2 in range(2)]
                for pr in range(2):          # head pair (2pr, 2pr+1)
                    ps_ctxA = _mk(psp, [HD + 1, SQ], F32, "ps")
                    ps_ctxB = _mk(psp, [HD + 1, SQ], F32, "ps")
                    nks = 4 * q + 4   # causally relevant sk tiles
                    firstA = firstB = True
                    for jk in range(nks):
                        dlt = jk - 4 * q
                        c0 = max(dlt, 0) * P  # cols < c0 fully masked
                        jc = (jk % 4) * P
                        ktq = kt2[jk // 4]
                        # two concurrent row-tiled score MMs (K=64 each)
                        # into the two banks of one pair tile
                        ssp = _mk(psp, [P, 2 * SQ], F32, "ss", bufs=2)
                        nc.tensor.matmul(
                            ssp[:, c0:SQ], ktq[:HD, jc:jc + P],
                            qpair[pr][q][:HD, c0:], start=True, stop=True)
                        nc.tensor.matmul(
                            ssp[:, SQ + c0:], ktq[HD:, jc:jc + P],
                            qpair[pr][q][HD:, c0:], start=True, stop=True)
                        # one exp for both heads (strided 2-region view)
                        pr2 = _mk(prp, [P, 2 * SQ], BF16, "pr")
                        nc.scalar.activation(
                            _v(pr2, [[SQ, 2], [1, SQ - c0]], extra_offset=c0),
                            _v(ssp, [[SQ, 2], [1, SQ - c0]], extra_offset=c0),
                            mybir.ActivationFunctionType.Exp, scale=SCALE)
                        last = (jk == nks - 1)
                        if dlt >= 0:
                            # masked diagonal block -> tri-multiplied copy;
                            # the mask-free remainder goes straight to the
                            # ctx MMs (keeps the DVE off the critical path)
                            prm = _mk(prp, [P, 2 * P], BF16, "prm")
                            nc.vector.tensor_mul(
                                _v(prm, [[P, 2], [1, P]]),
                                _v(pr2, [[SQ, 2], [1, P]], extra_offset=c0),
                                _v(tri_sb, [[0, 2], [1, P]]))
                            if c0 + P < SQ:
                                nc.tensor.matmul(
                                    ps_ctxA[:, c0 + P:], vs[jk],
                                    pr2[:, c0 + P:SQ],
                                    start=firstA, stop=False)
                                firstA = False
                                nc.tensor.matmul(
                                    ps_ctxB[:, c0 + P:], vs[jk],
                                    pr2[:, SQ + c0 + P:],
                                    start=firstB, stop=False)
                                firstB = False
                            nc.tensor.matmul(
                                ps_ctxA[:, c0:c0 + P], vs[jk], prm[:, :P],
                                start=firstA, stop=last)
                            firstA = False
                            nc.tensor.matmul(
                                ps_ctxB[:, c0:c0 + P], vs[jk], prm[:, P:],
                                start=firstB, stop=last)
                            firstB = False
                        else:
                            nc.tensor.matmul(ps_ctxA, vs[jk], pr2[:, :SQ],
                                             start=firstA, stop=False)
                            firstA = False
                            nc.tensor.matmul(ps_ctxB, vs[jk], pr2[:, SQ:],
                                             start=firstB, stop=False)
                            firstB = False
                    stashes = [stash_ctx(2 * pr, q, ps_ctxA),
                               stash_ctx(2 * pr + 1, q, ps_ctxB)]
                    for d in pend_den:
                        emit_den(*d)
                    pend_den = stashes
                    # out-proj filler keeps the PE saturated while the
                    # ACT engine grinds through the exps
                    for _ in range(4):
                        if pend_chunks:
                            op_chunk(*pend_chunks.pop(0))
            for d in pend_den:
                emit_den(*d)
            for ch in pend_chunks:
                op_chunk(*ch)
            for jj in range(4):
                for n2 in range(2):
                    op_chunk(NSQ - 1, jj, n2)

    nc.compile()
    return nc


def prep_inputs(x, cos, sin, Wq, Wk, Wv, Wo, q_norm_w, k_norm_w):
    """Host-side shard + layout prep. Returns per-core input maps."""
    xtn = np.ascontiguousarray(x.reshape(S, D).T).astype(NBF)

    # rope tables with (1 + norm_w) folded in, k-block appended, and the
    # sin first-half pre-negated (so rope is out = q*cos5 + rot(q)*sin5
    # with rot(q) = [q2, q1])
    half = HD // 2
    wq1 = (1.0 + q_norm_w).astype(np.float32)
    wk1 = (1.0 + k_norm_w).astype(np.float32)

    def rotw(w):
        return np.concatenate([w[half:], w[:half]])

    sin_m = sin.copy()
    sin_m[:, :half] = -sin_m[:, :half]
    cos_q = cos * wq1
    cos_k = cos * wk1
    sin_q = sin_m * rotw(wq1)
    sin_k = sin_m * rotw(wk1)
    cos5 = np.concatenate([np.tile(cos_q, (1, NHL)), cos_k], axis=1)
    sin5 = np.concatenate([np.tile(sin_q, (1, NHL)), sin_k], axis=1)
    cos5 = np.ascontiguousarray(cos5).astype(NBF)
    sin5 = np.ascontiguousarray(sin5).astype(NBF)

    tri = np.triu(np.ones((P, P), dtype=np.float32)).astype(NBF)  # p<=f

    Wqh = Wq.reshape(H, 2 * HD, D)
    in_maps = []
    for c in range(NCORE):
        hs = slice(NHL * c, NHL * (c + 1))
        wq_c = Wqh[hs, :HD, :].reshape(EL, D)       # q rows, 4 heads
        wgt_c = Wqh[hs, HD:, :].reshape(EL, D)      # gate rows
        wk_c = Wk[HD * c:HD * (c + 1), :]
        wv_c = Wv[HD * c:HD * (c + 1), :]
        wqkv_c = np.concatenate([wq_c, wk_c, wv_c], axis=0)  # [384, D]
        in_maps.append({
            "xt": xtn,
            "wqkv": np.ascontiguousarray(wqkv_c.T).astype(NBF),
            "wg": np.ascontiguousarray(wgt_c.T).astype(NBF),
            "wo": np.ascontiguousarray(
                Wo[:, EL * c:EL * (c + 1)].T).astype(NBF),
            "cos5": cos5,
            "sin5": sin5,
            "tri": tri,
        })
    return in_maps


_NC_CACHE = {}


def get_nc():
    if "nc" not in _NC_CACHE:
        _NC_CACHE["nc"] = build_nc()
    return _NC_CACHE["nc"]


def run(in_maps, trace=False, **kw):
    nc = get_nc()
    return run_bass_kernel_spmd(nc, in_maps, list(range(NCORE)),
                                trace=trace, **kw)


def kernel(x, mask, cos, sin, Wq, Wk, Wv, Wo, q_norm_w, k_norm_w):
    in_maps = prep_inputs(np.asarray(x, dtype=np.float32), np.asarray(cos),
                          np.asarray(sin), np.asarray(Wq), np.asarray(Wk),
                          np.asarray(Wv), np.asarray(Wo),
                          np.asarray(q_norm_w), np.asarray(k_norm_w))
    res = run(in_maps)
    acc = np.zeros((S, D), dtype=np.float32)
    for r in res.results:
        acc += np.asarray(r["out"], dtype=np.float32)
    return acc.reshape(1, S, D)


# revision 40
# speedup vs baseline: 1.0446x; 1.0446x over previous
"""GQA attention block (B=1, S=2048, D=2048, H=32, G=8, HD=64) on 8 trn2 cores.

Sharding: tensor-parallel over heads/KV-groups. Core c owns q-heads
4c..4c+3 and KV group c. Wq/Wk/Wv column-parallel, Wo row-parallel;
each core computes a partial [S, D] output, host sums the 8 partials.

Per-core dataflow (all matmuls bf16, stats f32), software-pipelined so
PE / ACT / DVE overlap and the PE rarely idles long enough for the HAM
clock gate to re-throttle it:
  Phase A: gate proj ([e,s] layout), i-outer accumulation paced by
    the streaming xt/wg DMAs; sigmoid via tanh.
  Phase B: qkv proj -> psum [s,384] per s-tile j, in groups of 4 with
    the RMS stats batched into one [128,20] Newton chain; RoPE on DVE
    (bf16, 2x mode); psum->sbuf staging copies ride the ACT engine.
    PE-transposes of group g-1 are interleaved after the MMs of group
    g so the PE never waits on the DVE chain.
  Phase C: attention, q-slice outer / head-PAIR inner. The two heads
    of a pair are computed by two concurrent row-tiled score MMs
    (K=64 each, partition halves of kT/qT duplicated/packed) into the
    two banks of one [128,1024] psum pair tile; ONE exp covers both
    heads via a strided 2-region view. The causal boundary block is
    tri-multiplied into a side tile so the mask-free remainder feeds
    the ctx MMs without a DVE hop. den goes through a lane-spread
    reciprocal ([1,512] DMA-reshaped to [128,4]), is PE-broadcast one
    pair late, and out-proj chunks are interleaved as PE filler.
    Partials written bf16, host sums the 8 cores.
"""

import numpy as np
import ml_dtypes

import concourse.bass as bass
import concourse.tile as tile
from concourse import bacc, mybir
from concourse.bass_utils import run_bass_kernel_spmd
from concourse.masks import make_identity

BF16 = mybir.dt.bfloat16
F32 = mybir.dt.float32
NBF = ml_dtypes.bfloat16

S = 2048
D = 2048
H = 32
G = 8
HD = 64
NCORE = 8
NHL = H // NCORE          # 4 q heads per core
EL = NHL * HD             # 256 local q (and gate, and ctx) features
QK = EL + HD              # 320: q + k features
QKV = QK + HD             # 384: q + k + v
P = 128
NS = S // P               # 16 s-tiles
ND = D // P               # 16 d-tiles
SQ = 512
NSQ = S // SQ             # 4 sq slices
NB = QK // HD             # 5 (hd,) blocks in the q|k strip
SCALE = HD ** -0.5
EPS = 1e-6


def _v(ap, dims, extra_offset=0):
    """Reshape the free dims of a 2D AP into `dims` ([step, count] pairs),
    keeping the partition dim."""
    return bass.AP(
        tensor=ap.tensor,
        offset=ap.offset + extra_offset,
        ap=[list(ap.ap[0])] + [list(d) for d in dims],
    )


def _mk(pool, shape, dtype, tag, bufs=None):
    return pool.tile(shape, dtype, tag=tag, name=tag, bufs=bufs)


def build_nc():
    nc = bacc.Bacc("TRN2", target_bir_lowering=False, debug=False,
                   num_devices=NCORE)

    xt = nc.dram_tensor("xt", [D, S], BF16, kind="ExternalInput").ap()
    wqkv = nc.dram_tensor("wqkv", [D, QKV], BF16, kind="ExternalInput").ap()
    wg = nc.dram_tensor("wg", [D, EL], BF16, kind="ExternalInput").ap()
    wo = nc.dram_tensor("wo", [EL, D], BF16, kind="ExternalInput").ap()
    cos5 = nc.dram_tensor("cos5", [S, QK], BF16, kind="ExternalInput").ap()
    sin5 = nc.dram_tensor("sin5", [S, QK], BF16, kind="ExternalInput").ap()
    tri = nc.dram_tensor("tri", [P, P], BF16, kind="ExternalInput").ap()
    out = nc.dram_tensor("out", [S, D], BF16, kind="ExternalOutput").ap()

    with tile.TileContext(nc) as tc:
        with (
            tc.tile_pool(name="persist", bufs=1) as pp,
            tc.tile_pool(name="work", bufs=2) as wp,
            tc.tile_pool(name="stats", bufs=3) as sp,
            tc.tile_pool(name="probs", bufs=4) as prp,
            tc.tile_pool(name="outc", bufs=4) as ocp,
            tc.tile_pool(name="psum", bufs=4, space="PSUM") as psp,
        ):
            # ---- persistent loads: wg+xt first (phase A is paced by
            # these), then wqkv (phase B), cos/sin, tri, wo ----
            qeng = [nc.sync, nc.scalar, nc.sync, nc.scalar]
            xts, wgs = [], []
            for i in range(ND):
                tg = _mk(pp, [P, EL], BF16, f"wg{i}")
                qeng[i % 4].dma_start(out=tg, in_=wg[i * P:(i + 1) * P, :])
                wgs.append(tg)
                t = _mk(pp, [P, S], BF16, f"xt{i}")
                qeng[(i + 1) % 4].dma_start(out=t,
                                            in_=xt[i * P:(i + 1) * P, :])
                xts.append(t)
            wqkvs = []
            for i in range(ND):
                t = _mk(pp, [P, QKV], BF16, f"wqkv{i}")
                qeng[i % 4].dma_start(out=t, in_=wqkv[i * P:(i + 1) * P, :])
                wqkvs.append(t)
            coss, sins = [], []
            for j in range(NS):
                tc_ = _mk(pp, [P, QK], BF16, f"cos{j}")
                nc.sync.dma_start(out=tc_, in_=cos5[j * P:(j + 1) * P, :])
                coss.append(tc_)
                ts_ = _mk(pp, [P, QK], BF16, f"sin{j}")
                nc.sync.dma_start(out=ts_, in_=sin5[j * P:(j + 1) * P, :])
                sins.append(ts_)
            tri_sb = _mk(pp, [P, P], BF16, "tri")
            nc.sync.dma_start(out=tri_sb, in_=tri)
            wos = []
            for e in range(2):
                t = _mk(pp, [P, D], BF16, f"wo{e}")
                nc.sync.dma_start(out=t, in_=wo[e * P:(e + 1) * P, :])
                wos.append(t)
            ident = _mk(pp, [P, P], BF16, "ident")
            make_identity(nc, ident)
            ones_b = _mk(pp, [1, P], BF16, "ones_b")
            nc.vector.memset(ones_b, 1.0)

            # persistent intermediate tensors
            # qpair[p][q]: rows 0-63 = qT of head 2p, 64-127 = head 2p+1
            # kt2[q]: kT duplicated on both partition halves (for the
            # row-tiled concurrent score matmuls)
            qpair = [[_mk(pp, [P, SQ], BF16, f"qp{p}_{q}")
                      for q in range(NSQ)] for p in range(2)]
            kt2 = [_mk(pp, [P, SQ], BF16, f"kt{q}") for q in range(NSQ)]
            vs = [_mk(pp, [P, HD + 1], BF16, f"v{j}") for j in range(NS)]
            for j in range(NS):
                nc.vector.memset(vs[j][:, HD:HD + 1], 1.0)
            gus2 = [[_mk(pp, [P, 2 * SQ], BF16, f"gu{p}_{qq}")
                     for qq in range(2)] for p in range(2)]
            ctxgs = [[_mk(pp, [P, SQ], BF16, f"cg{p}_{q}") for q in range(NSQ)]
                     for p in range(2)]

            # ---- phase A: gate projection, i-outer (DMA-paced) ----
            # p=0 accumulates into two 2-bank "ss" pair tiles, p=1 into
            # four 1-bank "ps" tiles (8 banks total live).
            ss_gs = [_mk(psp, [P, 2 * SQ], F32, "ss", bufs=2)
                     for _ in range(2)]
            ps_gs = [_mk(psp, [P, SQ], F32, "ps", bufs=4) for _ in range(4)]
            for i in range(ND):
                for q in range(NSQ):
                    nc.tensor.matmul(
                        ss_gs[q // 2][:, (q % 2) * SQ:(q % 2 + 1) * SQ],
                        wgs[i][:, 0:P], xts[i][:, q * SQ:(q + 1) * SQ],
                        start=(i == 0), stop=(i == ND - 1))
                for q in range(NSQ):
                    nc.tensor.matmul(
                        ps_gs[q], wgs[i][:, P:2 * P],
                        xts[i][:, q * SQ:(q + 1) * SQ],
                        start=(i == 0), stop=(i == ND - 1))
            # u = 1 + tanh(g/2) = 2*sigmoid(g)
            for qq in range(2):
                nc.scalar.activation(gus2[0][qq], ss_gs[qq],
                                     mybir.ActivationFunctionType.Tanh,
                                     scale=0.5)
            for q in range(NSQ):
                nc.scalar.activation(
                    gus2[1][q // 2][:, (q % 2) * SQ:(q % 2 + 1) * SQ],
                    ps_gs[q], mybir.ActivationFunctionType.Tanh, scale=0.5)
            for p in range(2):
                for qq in range(2):
                    nc.vector.tensor_scalar_add(gus2[p][qq], gus2[p][qq], 1.0)

            # ---- phase B: qkv projection + norm + rope + transpose ----
            # Processed in groups of 4 s-tiles: the RMS stats of a group
            # are batched into one [128, 20] Newton chain (tiny DVE ops
            # amortize 4x). PE transposes for group g-1 are emitted after
            # the MMs of group g, so the PE never waits on the DVE chain.
            pend_tr = None   # (j, qkr tile)

            def emit_transposes(j, qkr):
                jq, jc = j // 4, (j % 4) * P
                # all three transposes share one 2-bank "ss" tile (bf16,
                # all regions within bank 0's byte range is fine: each
                # matmul output stays inside one bank)
                tt = _mk(psp, [P, 4 * SQ], BF16, "ss", bufs=2)
                for p in range(2):
                    nc.tensor.transpose(tt[:, p * P:(p + 1) * P],
                                        qkr[:, p * P:(p + 1) * P], ident)
                    nc.scalar.copy(qpair[p][jq][:, jc:jc + P],
                                   tt[:, p * P:(p + 1) * P])
                nc.tensor.transpose(tt[:HD, 2 * P:3 * P],
                                    qkr[:, 2 * P:2 * P + HD], ident)
                nc.scalar.copy(kt2[jq][:HD, jc:jc + P],
                               tt[:HD, 2 * P:3 * P])
                # duplicate k onto the upper partition half via SBUF->SBUF
                # DMA (partition-crossing moves are DMA-only)
                nc.sync.dma_start(out=kt2[jq][HD:, jc:jc + P],
                                  in_=kt2[jq][:HD, jc:jc + P])

            GJ = 4                       # s-tiles per stats group
            pend_trs = []                # [(j, qkr), ...] of group g-1
            for g in range(NS // GJ):
                js = list(range(GJ * g, GJ * (g + 1)))
                qksbs = []
                ssg = _mk(sp, [P, NB * GJ], F32, "ssg")
                for gi, j in enumerate(js):
                    ps_qkv = _mk(psp, [P, QKV], F32, "ps")
                    for i in range(ND):
                        nc.tensor.matmul(
                            ps_qkv, xts[i][:, j * P:(j + 1) * P], wqkvs[i],
                            start=(i == 0), stop=(i == ND - 1))
                    if pend_trs:
                        emit_transposes(*pend_trs.pop(0))
                    # v straight to SBUF (ones column preset in prologue);
                    # psum->sbuf copies ride the otherwise-idle ACT engine
                    nc.scalar.copy(vs[j][:, :HD], ps_qkv[:, QK:QKV])
                    # park q|k in SBUF (bf16) and batch the RMS stats
                    qksb = _mk(wp, [P, QK], BF16, "qksb", bufs=2 * GJ)
                    nc.scalar.copy(qksb, ps_qkv[:, :QK])
                    qksbs.append(qksb)
                    sqr = _mk(wp, [P, QK], F32, "sqr")
                    nc.scalar.activation(sqr, qksb,
                                         mybir.ActivationFunctionType.Square)
                    nc.vector.tensor_reduce(
                        ssg[:, NB * gi:NB * (gi + 1)],
                        _v(sqr, [[HD, NB], [1, HD]]),
                        axis=mybir.AxisListType.X, op=mybir.AluOpType.add)
                # m = mean + eps;  r = rsqrt(m): poly seed + 2 Newton steps
                NBG = NB * GJ
                m = _mk(sp, [P, NBG], F32, "m")
                nc.vector.tensor_scalar(m, ssg, 1.0 / HD, EPS,
                                        mybir.AluOpType.mult,
                                        mybir.AluOpType.add)
                mc = _mk(sp, [P, NBG], F32, "mc")
                nc.vector.tensor_scalar(mc, m, 5.5, 0.45,
                                        mybir.AluOpType.min,
                                        mybir.AluOpType.max)
                m2 = _mk(sp, [P, NBG], F32, "m2")
                nc.vector.tensor_mul(m2, mc, mc)
                lin = _mk(sp, [P, NBG], F32, "lin")
                nc.vector.tensor_scalar(lin, mc, -0.48330447, 1.51774376,
                                        mybir.AluOpType.mult,
                                        mybir.AluOpType.add)
                y = _mk(sp, [P, NBG], F32, "y")
                nc.vector.scalar_tensor_tensor(y, m2, 0.0534932, lin,
                                               mybir.AluOpType.mult,
                                               mybir.AluOpType.add)
                ytmp = _mk(sp, [P, NBG], F32, "ytmp")
                for _ in range(2):
                    nc.vector.tensor_mul(ytmp, y, y)              # y^2
                    nc.vector.tensor_mul(ytmp, ytmp, m)           # m y^2
                    nc.vector.tensor_scalar(ytmp, ytmp, -0.5, 1.5,
                                            mybir.AluOpType.mult,
                                            mybir.AluOpType.add)
                    nc.vector.tensor_mul(y, y, ytmp)
                for gi, j in enumerate(js):
                    # qkn = qk * r (broadcast r over hd)
                    qkn = _mk(wp, [P, QK], BF16, "qkn")
                    nc.vector.tensor_mul(
                        _v(qkn, [[HD, NB], [1, HD]]),
                        _v(qksbs[gi], [[HD, NB], [1, HD]]),
                        _v(y[:, NB * gi:], [[1, NB], [0, HD]]))
                    # rope: out = qkn*cos5 + rot(qkn)*sin5  (sin pre-negated
                    # on the first half on host; cos/sin include 1+norm_w)
                    t1 = _mk(wp, [P, QK], BF16, "t1")
                    nc.vector.tensor_mul(t1, qkn, coss[j])
                    t2 = _mk(wp, [P, QK], BF16, "t2")
                    rot = _v(qkn[:, :], [[HD, NB], [-32, 2], [1, 32]],
                             extra_offset=32)
                    nc.vector.tensor_mul(
                        _v(t2, [[HD, NB], [32, 2], [1, 32]]), rot,
                        _v(sins[j][:, :], [[HD, NB], [32, 2], [1, 32]]))
                    qkr = _mk(wp, [P, QK], BF16, "qkr", bufs=2 * GJ)
                    nc.vector.tensor_add(qkr, t1, t2)
                    pend_trs.append((j, qkr))
            for tr in pend_trs:
                emit_transposes(*tr)

            # ---- phase C: attention, q-slice outer / head inner, with
            # delayed den processing and interleaved out-projection ----
            def stash_ctx(h, q, ps_ctx):
                """Emitted right after the jk loop: start the lane-spread
                reciprocal (den [1,512] is DMA-reshaped to [128,4] so the
                iterative-divide runs 128 lanes wide instead of 1), then
                DMA-broadcast 0.5/den (bf16) across 64 partitions."""
                dsb = _mk(sp, [1, SQ], F32, "dsb", bufs=3)
                nc.scalar.copy(dsb, ps_ctx[HD:HD + 1, :])
                dstk = _mk(sp, [P, SQ // P], F32, "dstk", bufs=3)
                nc.sync.dma_start(out=dstk, in_=dsb)
                rsm = _mk(sp, [P, SQ // P], F32, "rsm", bufs=3)
                nc.vector.reciprocal(rsm, dstk)
                rsmb = _mk(sp, [P, SQ // P], BF16, "rsmb", bufs=3)
                nc.vector.tensor_scalar_mul(rsmb, rsm, 0.5)
                denr = _mk(sp, [1, SQ], BF16, "denrb", bufs=3)
                nc.sync.dma_start(out=denr, in_=rsmb)
                return (h, q, ps_ctx, denr)

            def emit_den(h, q, ps_ctx, denr):
                """Delayed one head-pair: PE broadcast of 0.5/den + the
                gate/den multiplies."""
                hp, ho = h // 2, (h % 2) * HD
                ps_db = _mk(psp, [P, SQ], F32, "ps")
                nc.tensor.matmul(ps_db, ones_b, denr, start=True, stop=True)
                # ctxg = ctx * u * (0.5/den)
                tmpu = _mk(wp, [HD, SQ], BF16, "tmpu")
                nc.vector.tensor_mul(
                    tmpu, ps_ctx[:HD, :],
                    gus2[hp][q // 2][ho:ho + HD,
                                     (q % 2) * SQ:(q % 2) * SQ + SQ])
                nc.vector.tensor_mul(
                    ctxgs[hp][q][ho:ho + HD, :], tmpu, ps_db[:HD, :])

            def op_chunk(jq, jj, n2):
                j = 4 * jq + jj
                jc = jj * P
                ps_o = _mk(psp, [P, 2 * SQ], F32, "ss", bufs=2)
                for half in range(2):
                    n = 2 * n2 + half
                    for e in range(2):
                        nc.tensor.matmul(
                            ps_o[:, half * SQ:(half + 1) * SQ],
                            ctxgs[e][jq][:, jc:jc + P],
                            wos[e][:, n * SQ:(n + 1) * SQ],
                            start=(e == 0), stop=(e == 1))
                oc = _mk(ocp, [P, 2 * SQ], BF16, "oc")
                nc.vector.tensor_copy(out=oc, in_=ps_o)
                nc.sync.dma_start(
                    out=out[j * P:(j + 1) * P,
                            n2 * 2 * SQ:(n2 + 1) * 2 * SQ],
                    in_=oc)

            pend_den = []      # stashes of the previous head-pair
            pend_chunks = []   # deferred out-proj (jq, jj, n) chunks
            for q in range(NSQ):
                jq = q - 1
                if jq >= 0:
                    pend_chunks += [(jq, jj, n2) for jj in range(4)
                                    for n2 in range(2)]
                for pr in range(2):          # head pair (2pr, 2pr+1)
                    ps_ctxA = _mk(psp, [HD + 1, SQ], F32, "ps")
                    ps_ctxB = _mk(psp, [HD + 1, SQ], F32, "ps")
                    nks = 4 * q + 4   # causally relevant sk tiles
                    firstA = firstB = True
                    for jk in range(nks):
                        dlt = jk - 4 * q
                        c0 = max(dlt, 0) * P  # cols < c0 fully masked
                        jc = (jk % 4) * P
                        ktq = kt2[jk // 4]
                        # two concurrent row-tiled score MMs (K=64 each)
                        # into the two banks of one pair tile
                        ssp = _mk(psp, [P, 2 * SQ], F32, "ss", bufs=2)
                        nc.tensor.matmul(
                            ssp[:, c0:SQ], ktq[:HD, jc:jc + P],
                            qpair[pr][q][:HD, c0:], start=True, stop=True)
                        nc.tensor.matmul(
                            ssp[:, SQ + c0:], ktq[HD:, jc:jc + P],
                            qpair[pr][q][HD:, c0:], start=True, stop=True)
                        # one exp for both heads (strided 2-region view)
                        pr2 = _mk(prp, [P, 2 * SQ], BF16, "pr")
                        nc.scalar.activation(
                            _v(pr2, [[SQ, 2], [1, SQ - c0]], extra_offset=c0),
                            _v(ssp, [[SQ, 2], [1, SQ - c0]], extra_offset=c0),
                            mybir.ActivationFunctionType.Exp, scale=SCALE)
                        last = (jk == nks - 1)
                        if dlt >= 0:
                            # masked diagonal block -> tri-multiplied copy;
                            # the mask-free remainder goes straight to the
                            # ctx MMs (keeps the DVE off the critical path)
                            prm = _mk(prp, [P, 2 * P], BF16, "prm")
                            nc.vector.tensor_mul(
                                _v(prm, [[P, 2], [1, P]]),
                                _v(pr2, [[SQ, 2], [1, P]], extra_offset=c0),
                                _v(tri_sb, [[0, 2], [1, P]]))
                            if c0 + P < SQ:
                                nc.tensor.matmul(
                                    ps_ctxA[:, c0 + P:], vs[jk],
                                    pr2[:, c0 + P:SQ],
                                    start=firstA, stop=False)
                                firstA = False
                                nc.tensor.matmul(
                                    ps_ctxB[:, c0 + P:], vs[jk],
                                    pr2[:, SQ + c0 + P:],
                                    start=firstB, stop=False)
                                firstB = False
                            nc.tensor.matmul(
                                ps_ctxA[:, c0:c0 + P], vs[jk], prm[:, :P],
                                start=firstA, stop=last)
                            firstA = False
                            nc.tensor.matmul(
                                ps_ctxB[:, c0:c0 + P], vs[jk], prm[:, P:],
                                start=firstB, stop=last)
                            firstB = False
                        else:
                            nc.tensor.matmul(ps_ctxA, vs[jk], pr2[:, :SQ],
                                             start=firstA, stop=False)
                            firstA = False
                            nc.tensor.matmul(ps_ctxB, vs[jk], pr2[:, SQ:],
                                             start=firstB, stop=False)
                            firstB = False
                    stashes = [stash_ctx(2 * pr, q, ps_ctxA),
                               stash_ctx(2 * pr + 1, q, ps_ctxB)]
                    for d in pend_den:
                        emit_den(*d)
                    pend_den = stashes
                    # out-proj filler keeps the PE saturated while the
                    # ACT engine grinds through the exps
                    for _ in range(4):
                        if pend_chunks:
                            op_chunk(*pend_chunks.pop(0))
            for d in pend_den:
                emit_den(*d)
            for ch in pend_chunks:
                op_chunk(*ch)
            for jj in range(4):
                for n2 in range(2):
                    op_chunk(NSQ - 1, jj, n2)

    nc.compile()
    return nc


def prep_inputs(x, cos, sin, Wq, Wk, Wv, Wo, q_norm_w, k_norm_w):
    """Host-side shard + layout prep. Returns per-core input maps."""
    xtn = np.ascontiguousarray(x.reshape(S, D).T).astype(NBF)

    # rope tables with (1 + norm_w) folded in, k-block appended, and the
    # sin first-half pre-negated (so rope is out = q*cos5 + rot(q)*sin5
    # with rot(q) = [q2, q1])
    half = HD // 2
    wq1 = (1.0 + q_norm_w).astype(np.float32)
    wk1 = (1.0 + k_norm_w).astype(np.float32)

    def rotw(w):
        return np.concatenate([w[half:], w[:half]])

    sin_m = sin.copy()
    sin_m[:, :half] = -sin_m[:, :half]
    cos_q = cos * wq1
    cos_k = cos * wk1
    sin_q = sin_m * rotw(wq1)
    sin_k = sin_m * rotw(wk1)
    cos5 = np.concatenate([np.tile(cos_q, (1, NHL)), cos_k], axis=1)
    sin5 = np.concatenate([np.tile(sin_q, (1, NHL)), sin_k], axis=1)
    cos5 = np.ascontiguousarray(cos5).astype(NBF)
    sin5 = np.ascontiguousarray(sin5).astype(NBF)

    tri = np.triu(np.ones((P, P), dtype=np.float32)).astype(NBF)  # p<=f

    Wqh = Wq.reshape(H, 2 * HD, D)
    in_maps = []
    for c in range(NCORE):
        hs = slice(NHL * c, NHL * (c + 1))
        wq_c = Wqh[hs, :HD, :].reshape(EL, D)       # q rows, 4 heads
        wgt_c = Wqh[hs, HD:, :].reshape(EL, D)      # gate rows
        wk_c = Wk[HD * c:HD * (c + 1), :]
        wv_c = Wv[HD * c:HD * (c + 1), :]
        wqkv_c = np.concatenate([wq_c, wk_c, wv_c], axis=0)  # [384, D]
        in_maps.append({
            "xt": xtn,
            "wqkv": np.ascontiguousarray(wqkv_c.T).astype(NBF),
            "wg": np.ascontiguousarray(wgt_c.T).astype(NBF),
            "wo": np.ascontiguousarray(
                Wo[:, EL * c:EL * (c + 1)].T).astype(NBF),
            "cos5": cos5,
            "sin5": sin5,
            "tri": tri,
        })
    return in_maps


_NC_CACHE = {}


def get_nc():
    if "nc" not in _NC_CACHE:
        _NC_CACHE["nc"] = build_nc()
    return _NC_CACHE["nc"]


def run(in_maps, trace=False, **kw):
    nc = get_nc()
    return run_bass_kernel_spmd(nc, in_maps, list(range(NCORE)),
                                trace=trace, **kw)


def kernel(x, mask, cos, sin, Wq, Wk, Wv, Wo, q_norm_w, k_norm_w):
    in_maps = prep_inputs(np.asarray(x, dtype=np.float32), np.asarray(cos),
                          np.asarray(sin), np.asarray(Wq), np.asarray(Wk),
                          np.asarray(Wv), np.asarray(Wo),
                          np.asarray(q_norm_w), np.asarray(k_norm_w))
    res = run(in_maps)
    acc = np.zeros((S, D), dtype=np.float32)
    for r in res.results:
        acc += np.asarray(r["out"], dtype=np.float32)
    return acc.reshape(1, S, D)


# revision 44
# speedup vs baseline: 1.0586x; 1.0133x over previous
"""GQA attention block (B=1, S=2048, D=2048, H=32, G=8, HD=64) on 8 trn2 cores.

Sharding: tensor-parallel over heads/KV-groups. Core c owns q-heads
4c..4c+3 and KV group c. Wq/Wk/Wv column-parallel, Wo row-parallel;
each core computes a partial [S, D] output, host sums the 8 partials.

Per-core dataflow (all matmuls bf16, stats f32), software-pipelined so
PE / ACT / DVE overlap and the PE rarely idles long enough for the HAM
clock gate to re-throttle it:
  Phase A: gate proj ([e,s] layout), i-outer accumulation paced by
    the streaming xt/wg DMAs; sigmoid via tanh.
  Phase B: qkv proj -> psum [s,384] per s-tile j, in groups of 4 with
    the RMS stats batched into one [128,20] Newton chain; RoPE on DVE
    (bf16, 2x mode); psum->sbuf staging copies ride the ACT engine.
    PE-transposes of group g-1 are interleaved after the MMs of group
    g so the PE never waits on the DVE chain.
  Phase C: attention, q-slice outer / head-PAIR inner. The two heads
    of a pair are computed by two concurrent row-tiled score MMs
    (K=64 each, partition halves of kT/qT duplicated/packed) into the
    two banks of one [128,1024] psum pair tile; ONE exp covers both
    heads via a strided 2-region view. The causal boundary block is
    tri-multiplied into a side tile so the mask-free remainder feeds
    the ctx MMs without a DVE hop. den goes through a lane-spread
    reciprocal ([1,512] DMA-reshaped to [128,4]), is PE-broadcast one
    pair late, and out-proj chunks are interleaved as PE filler.
    Partials written bf16, host sums the 8 cores.
"""

import numpy as np
import ml_dtypes

import concourse.bass as bass
import concourse.tile as tile
from concourse import bacc, mybir
from concourse.bass_utils import run_bass_kernel_spmd
from concourse.masks import make_identity

BF16 = mybir.dt.bfloat16
F32 = mybir.dt.float32
NBF = ml_dtypes.bfloat16

S = 2048
D = 2048
H = 32
G = 8
HD = 64
NCORE = 8
NHL = H // NCORE          # 4 q heads per core
EL = NHL * HD             # 256 local q (and gate, and ctx) features
QK = EL + HD              # 320: q + k features
QKV = QK + HD             # 384: q + k + v
P = 128
NS = S // P               # 16 s-tiles
ND = D // P               # 16 d-tiles
SQ = 512
NSQ = S // SQ             # 4 sq slices
NB = QK // HD             # 5 (hd,) blocks in the q|k strip
SCALE = HD ** -0.5
EPS = 1e-6


def _v(ap, dims, extra_offset=0):
    """Reshape the free dims of a 2D AP into `dims` ([step, count] pairs),
    keeping the partition dim."""
    return bass.AP(
        tensor=ap.tensor,
        offset=ap.offset + extra_offset,
        ap=[list(ap.ap[0])] + [list(d) for d in dims],
    )


def _mk(pool, shape, dtype, tag, bufs=None):
    return pool.tile(shape, dtype, tag=tag, name=tag, bufs=bufs)


def build_nc():
    nc = bacc.Bacc("TRN2", target_bir_lowering=False, debug=False,
                   num_devices=NCORE)

    xt = nc.dram_tensor("xt", [D, S], BF16, kind="ExternalInput").ap()
    wqkv = nc.dram_tensor("wqkv", [D, QKV], BF16, kind="ExternalInput").ap()
    wg = nc.dram_tensor("wg", [D, EL], BF16, kind="ExternalInput").ap()
    wo = nc.dram_tensor("wo", [EL, D], BF16, kind="ExternalInput").ap()
    cos5 = nc.dram_tensor("cos5", [S, QK], BF16, kind="ExternalInput").ap()
    sin5 = nc.dram_tensor("sin5", [S, QK], BF16, kind="ExternalInput").ap()
    tri = nc.dram_tensor("tri", [P, P], BF16, kind="ExternalInput").ap()
    out = nc.dram_tensor("out", [S, D], BF16, kind="ExternalOutput").ap()

    with tile.TileContext(nc) as tc:
        with (
            tc.tile_pool(name="persist", bufs=1) as pp,
            tc.tile_pool(name="work", bufs=2) as wp,
            tc.tile_pool(name="stats", bufs=3) as sp,
            tc.tile_pool(name="probs", bufs=4) as prp,
            tc.tile_pool(name="outc", bufs=4) as ocp,
            tc.tile_pool(name="psum", bufs=4, space="PSUM") as psp,
        ):
            # ---- persistent loads: wg+xt first (phase A is paced by
            # these), then wqkv (phase B), cos/sin, tri, wo ----
            qeng = [nc.sync, nc.scalar, nc.sync, nc.scalar]
            xts, wgs = [], []
            for i in range(ND):
                tg = _mk(pp, [P, EL], BF16, f"wg{i}")
                qeng[i % 4].dma_start(out=tg, in_=wg[i * P:(i + 1) * P, :])
                wgs.append(tg)
                t = _mk(pp, [P, S], BF16, f"xt{i}")
                qeng[(i + 1) % 4].dma_start(out=t,
                                            in_=xt[i * P:(i + 1) * P, :])
                xts.append(t)
            wqkvs = []
            for i in range(ND):
                t = _mk(pp, [P, QKV], BF16, f"wqkv{i}")
                qeng[i % 4].dma_start(out=t, in_=wqkv[i * P:(i + 1) * P, :])
                wqkvs.append(t)
            coss, sins = [], []
            for j in range(NS):
                tc_ = _mk(pp, [P, QK], BF16, f"cos{j}")
                nc.sync.dma_start(out=tc_, in_=cos5[j * P:(j + 1) * P, :])
                coss.append(tc_)
                ts_ = _mk(pp, [P, QK], BF16, f"sin{j}")
                nc.sync.dma_start(out=ts_, in_=sin5[j * P:(j + 1) * P, :])
                sins.append(ts_)
            tri_sb = _mk(pp, [P, P], BF16, "tri")
            nc.sync.dma_start(out=tri_sb, in_=tri)
            wos = []
            for e in range(2):
                t = _mk(pp, [P, D], BF16, f"wo{e}")
                nc.sync.dma_start(out=t, in_=wo[e * P:(e + 1) * P, :])
                wos.append(t)
            ident = _mk(pp, [P, P], BF16, "ident")
            make_identity(nc, ident)
            ones_b = _mk(pp, [1, P], BF16, "ones_b")
            nc.vector.memset(ones_b, 1.0)

            # persistent intermediate tensors
            # qpair[p][q]: rows 0-63 = qT of head 2p, 64-127 = head 2p+1
            # kt2[q]: kT duplicated on both partition halves (for the
            # row-tiled concurrent score matmuls)
            qpair = [[_mk(pp, [P, SQ], BF16, f"qp{p}_{q}")
                      for q in range(NSQ)] for p in range(2)]
            kt2 = [_mk(pp, [P, SQ], BF16, f"kt{q}") for q in range(NSQ)]
            vs = [_mk(pp, [P, HD + 1], BF16, f"v{j}") for j in range(NS)]
            for j in range(NS):
                nc.vector.memset(vs[j][:, HD:HD + 1], 1.0)
            gus2 = [[_mk(pp, [P, 2 * SQ], BF16, f"gu{p}_{qq}")
                     for qq in range(2)] for p in range(2)]
            ctxgs = [[_mk(pp, [P, SQ], BF16, f"cg{p}_{q}") for q in range(NSQ)]
                     for p in range(2)]

            # ---- phase A: gate projection, i-outer (DMA-paced) ----
            # p=0 accumulates into two 2-bank "ss" pair tiles, p=1 into
            # four 1-bank "ps" tiles (8 banks total live).
            ss_gs = [_mk(psp, [P, 2 * SQ], F32, "ss", bufs=2)
                     for _ in range(2)]
            ps_gs = [_mk(psp, [P, SQ], F32, "ps", bufs=4) for _ in range(4)]
            for i in range(ND):
                for q in range(NSQ):
                    nc.tensor.matmul(
                        ss_gs[q // 2][:, (q % 2) * SQ:(q % 2 + 1) * SQ],
                        wgs[i][:, 0:P], xts[i][:, q * SQ:(q + 1) * SQ],
                        start=(i == 0), stop=(i == ND - 1))
                for q in range(NSQ):
                    nc.tensor.matmul(
                        ps_gs[q], wgs[i][:, P:2 * P],
                        xts[i][:, q * SQ:(q + 1) * SQ],
                        start=(i == 0), stop=(i == ND - 1))
            # u = 1 + tanh(g/2) = 2*sigmoid(g).  The "ps"-tile tanhs run
            # first so phase B's qkv matmuls get their psum slots back
            # as early as possible.
            for q in range(NSQ):
                nc.scalar.activation(
                    gus2[1][q // 2][:, (q % 2) * SQ:(q % 2 + 1) * SQ],
                    ps_gs[q], mybir.ActivationFunctionType.Tanh, scale=0.5)
            for qq in range(2):
                nc.scalar.activation(gus2[0][qq], ss_gs[qq],
                                     mybir.ActivationFunctionType.Tanh,
                                     scale=0.5)
            for p in range(2):
                for qq in range(2):
                    nc.vector.tensor_scalar_add(gus2[p][qq], gus2[p][qq], 1.0)

            # ---- phase B: qkv projection + norm + rope + transpose ----
            # Processed in groups of 4 s-tiles: the RMS stats of a group
            # are batched into one [128, 20] Newton chain (tiny DVE ops
            # amortize 4x). PE transposes for group g-1 are emitted after
            # the MMs of group g, so the PE never waits on the DVE chain.
            pend_tr = None   # (j, qkr tile)

            def emit_transposes(j, qkr):
                jq, jc = j // 4, (j % 4) * P
                # all three transposes share one 2-bank "ss" tile (bf16,
                # all regions within bank 0's byte range is fine: each
                # matmul output stays inside one bank).
                # The last group's copies go to DVE so phase C's first
                # exps aren't queued behind a copy backlog on the FIFO
                # ACT engine (that stall re-throttled the PE clock).
                late = j >= NS - GJ
                tt = _mk(psp, [P, 4 * SQ], BF16, "ss", bufs=2)
                for p in range(2):
                    nc.tensor.transpose(tt[:, p * P:(p + 1) * P],
                                        qkr[:, p * P:(p + 1) * P], ident)
                    if late:
                        nc.vector.tensor_copy(
                            out=qpair[p][jq][:, jc:jc + P],
                            in_=tt[:, p * P:(p + 1) * P])
                    else:
                        nc.scalar.copy(qpair[p][jq][:, jc:jc + P],
                                       tt[:, p * P:(p + 1) * P])
                nc.tensor.transpose(tt[:HD, 2 * P:3 * P],
                                    qkr[:, 2 * P:2 * P + HD], ident)
                if late:
                    nc.vector.tensor_copy(out=kt2[jq][:HD, jc:jc + P],
                                          in_=tt[:HD, 2 * P:3 * P])
                else:
                    nc.scalar.copy(kt2[jq][:HD, jc:jc + P],
                                   tt[:HD, 2 * P:3 * P])
                # duplicate k onto the upper partition half via SBUF->SBUF
                # DMA (partition-crossing moves are DMA-only)
                nc.sync.dma_start(out=kt2[jq][HD:, jc:jc + P],
                                  in_=kt2[jq][:HD, jc:jc + P])

            GJ = 4                       # s-tiles per stats group
            pend_trs = []                # [(j, qkr), ...] of group g-1
            for g in range(NS // GJ):
                js = list(range(GJ * g, GJ * (g + 1)))
                qksbs = []
                ssg = _mk(sp, [P, NB * GJ], F32, "ssg")
                for gi, j in enumerate(js):
                    ps_qkv = _mk(psp, [P, QKV], F32, "ps")
                    for i in range(ND):
                        nc.tensor.matmul(
                            ps_qkv, xts[i][:, j * P:(j + 1) * P], wqkvs[i],
                            start=(i == 0), stop=(i == ND - 1))
                    if pend_trs:
                        emit_transposes(*pend_trs.pop(0))
                    # v straight to SBUF (ones column preset in prologue);
                    # psum->sbuf copies ride the otherwise-idle ACT engine
                    # (except the last group -- see emit_transposes)
                    qksb = _mk(wp, [P, QK], BF16, "qksb", bufs=2 * GJ)
                    if g == NS // GJ - 1:
                        nc.vector.tensor_copy(out=vs[j][:, :HD],
                                              in_=ps_qkv[:, QK:QKV])
                        nc.vector.tensor_copy(out=qksb, in_=ps_qkv[:, :QK])
                    else:
                        nc.scalar.copy(vs[j][:, :HD], ps_qkv[:, QK:QKV])
                        nc.scalar.copy(qksb, ps_qkv[:, :QK])
                    qksbs.append(qksb)
                    sqr = _mk(wp, [P, QK], F32, "sqr")
                    nc.scalar.activation(sqr, qksb,
                                         mybir.ActivationFunctionType.Square)
                    nc.vector.tensor_reduce(
                        ssg[:, NB * gi:NB * (gi + 1)],
                        _v(sqr, [[HD, NB], [1, HD]]),
                        axis=mybir.AxisListType.X, op=mybir.AluOpType.add)
                # m = mean + eps;  r = rsqrt(m): poly seed + 2 Newton steps
                NBG = NB * GJ
                m = _mk(sp, [P, NBG], F32, "m")
                nc.vector.tensor_scalar(m, ssg, 1.0 / HD, EPS,
                                        mybir.AluOpType.mult,
                                        mybir.AluOpType.add)
                mc = _mk(sp, [P, NBG], F32, "mc")
                nc.vector.tensor_scalar(mc, m, 5.5, 0.45,
                                        mybir.AluOpType.min,
                                        mybir.AluOpType.max)
                m2 = _mk(sp, [P, NBG], F32, "m2")
                nc.vector.tensor_mul(m2, mc, mc)
                lin = _mk(sp, [P, NBG], F32, "lin")
                nc.vector.tensor_scalar(lin, mc, -0.48330447, 1.51774376,
                                        mybir.AluOpType.mult,
                                        mybir.AluOpType.add)
                y = _mk(sp, [P, NBG], F32, "y")
                nc.vector.scalar_tensor_tensor(y, m2, 0.0534932, lin,
                                               mybir.AluOpType.mult,
                                               mybir.AluOpType.add)
                ytmp = _mk(sp, [P, NBG], F32, "ytmp")
                for _ in range(2):
                    nc.vector.tensor_mul(ytmp, y, y)              # y^2
                    nc.vector.tensor_mul(ytmp, ytmp, m)           # m y^2
                    nc.vector.tensor_scalar(ytmp, ytmp, -0.5, 1.5,
                                            mybir.AluOpType.mult,
                                            mybir.AluOpType.add)
                    nc.vector.tensor_mul(y, y, ytmp)
                for gi, j in enumerate(js):
                    # qkn = qk * r (broadcast r over hd)
                    qkn = _mk(wp, [P, QK], BF16, "qkn")
                    nc.vector.tensor_mul(
                        _v(qkn, [[HD, NB], [1, HD]]),
                        _v(qksbs[gi], [[HD, NB], [1, HD]]),
                        _v(y[:, NB * gi:], [[1, NB], [0, HD]]))
                    # rope: out = qkn*cos5 + rot(qkn)*sin5  (sin pre-negated
                    # on the first half on host; cos/sin include 1+norm_w)
                    t1 = _mk(wp, [P, QK], BF16, "t1")
                    nc.vector.tensor_mul(t1, qkn, coss[j])
                    t2 = _mk(wp, [P, QK], BF16, "t2")
                    rot = _v(qkn[:, :], [[HD, NB], [-32, 2], [1, 32]],
                             extra_offset=32)
                    nc.vector.tensor_mul(
                        _v(t2, [[HD, NB], [32, 2], [1, 32]]), rot,
                        _v(sins[j][:, :], [[HD, NB], [32, 2], [1, 32]]))
                    qkr = _mk(wp, [P, QK], BF16, "qkr", bufs=2 * GJ)
                    nc.vector.tensor_add(qkr, t1, t2)
                    pend_trs.append((j, qkr))
            for tr in pend_trs:
                emit_transposes(*tr)

            # ---- phase C: attention, q-slice outer / head inner, with
            # delayed den processing and interleaved out-projection ----
            def stash_ctx(h, q, ps_ctx):
                """Emitted right after the jk loop: start the lane-spread
                reciprocal (den [1,512] is DMA-reshaped to [128,4] so the
                iterative-divide runs 128 lanes wide instead of 1), then
                DMA-broadcast 0.5/den (bf16) across 64 partitions."""
                dsb = _mk(sp, [1, SQ], F32, "dsb", bufs=3)
                nc.vector.tensor_copy(out=dsb, in_=ps_ctx[HD:HD + 1, :])
                dstk = _mk(sp, [P, SQ // P], F32, "dstk", bufs=3)
                nc.sync.dma_start(out=dstk, in_=dsb)
                rsm = _mk(sp, [P, SQ // P], F32, "rsm", bufs=3)
                nc.vector.reciprocal(rsm, dstk)
                rsmb = _mk(sp, [P, SQ // P], BF16, "rsmb", bufs=3)
                nc.vector.tensor_scalar_mul(rsmb, rsm, 0.5)
                denr = _mk(sp, [1, SQ], BF16, "denrb", bufs=3)
                nc.sync.dma_start(out=denr, in_=rsmb)
                return (h, q, ps_ctx, denr)

            def emit_den(h, q, ps_ctx, denr):
                """Delayed one head-pair: PE broadcast of 0.5/den + the
                gate/den multiplies."""
                hp, ho = h // 2, (h % 2) * HD
                ps_db = _mk(psp, [P, SQ], F32, "ps")
                nc.tensor.matmul(ps_db, ones_b, denr, start=True, stop=True)
                # ctxg = ctx * u * (0.5/den)
                tmpu = _mk(wp, [HD, SQ], BF16, "tmpu")
                nc.vector.tensor_mul(
                    tmpu, ps_ctx[:HD, :],
                    gus2[hp][q // 2][ho:ho + HD,
                                     (q % 2) * SQ:(q % 2) * SQ + SQ])
                nc.vector.tensor_mul(
                    ctxgs[hp][q][ho:ho + HD, :], tmpu, ps_db[:HD, :])

            def op_chunk(jq, jj, n2):
                j = 4 * jq + jj
                jc = jj * P
                ps_o = _mk(psp, [P, 2 * SQ], F32, "ss", bufs=2)
                for half in range(2):
                    n = 2 * n2 + half
                    for e in range(2):
                        nc.tensor.matmul(
                            ps_o[:, half * SQ:(half + 1) * SQ],
                            ctxgs[e][jq][:, jc:jc + P],
                            wos[e][:, n * SQ:(n + 1) * SQ],
                            start=(e == 0), stop=(e == 1))
                oc = _mk(ocp, [P, 2 * SQ], BF16, "oc")
                nc.vector.tensor_copy(out=oc, in_=ps_o)
                nc.sync.dma_start(
                    out=out[j * P:(j + 1) * P,
                            n2 * 2 * SQ:(n2 + 1) * 2 * SQ],
                    in_=oc)

            pend_den = []      # stashes of the previous head-pair
            pend_chunks = []   # deferred out-proj (jq, jj, n) chunks
            for q in range(NSQ):
                jq = q - 1
                if jq >= 0:
                    pend_chunks += [(jq, jj, n2) for jj in range(4)
                                    for n2 in range(2)]
                for pr in range(2):          # head pair (2pr, 2pr+1)
                    ps_ctxA = _mk(psp, [HD + 1, SQ], F32, "ps")
                    ps_ctxB = _mk(psp, [HD + 1, SQ], F32, "ps")
                    nks = 4 * q + 4   # causally relevant sk tiles
                    firstA = firstB = True
                    for jk in range(nks):
                        dlt = jk - 4 * q
                        c0 = max(dlt, 0) * P  # cols < c0 fully masked
                        jc = (jk % 4) * P
                        ktq = kt2[jk // 4]
                        # two concurrent row-tiled score MMs (K=64 each)
                        # into the two banks of one pair tile
                        ssp = _mk(psp, [P, 2 * SQ], F32, "ss", bufs=2)
                        nc.tensor.matmul(
                            ssp[:, c0:SQ], ktq[:HD, jc:jc + P],
                            qpair[pr][q][:HD, c0:], start=True, stop=True)
                        nc.tensor.matmul(
                            ssp[:, SQ + c0:], ktq[HD:, jc:jc + P],
                            qpair[pr][q][HD:, c0:], start=True, stop=True)
                        # one exp for both heads (strided 2-region view)
                        pr2 = _mk(prp, [P, 2 * SQ], BF16, "pr")
                        nc.scalar.activation(
                            _v(pr2, [[SQ, 2], [1, SQ - c0]], extra_offset=c0),
                            _v(ssp, [[SQ, 2], [1, SQ - c0]], extra_offset=c0),
                            mybir.ActivationFunctionType.Exp, scale=SCALE)
                        last = (jk == nks - 1)
                        if dlt >= 0:
                            # masked diagonal block -> tri-multiplied copy;
                            # the mask-free remainder goes straight to the
                            # ctx MMs (keeps the DVE off the critical path)
                            prm = _mk(prp, [P, 2 * P], BF16, "prm")
                            nc.vector.tensor_mul(
                                _v(prm, [[P, 2], [1, P]]),
                                _v(pr2, [[SQ, 2], [1, P]], extra_offset=c0),
                                _v(tri_sb, [[0, 2], [1, P]]))
                            if c0 + P < SQ:
                                nc.tensor.matmul(
                                    ps_ctxA[:, c0 + P:], vs[jk],
                                    pr2[:, c0 + P:SQ],
                                    start=firstA, stop=False)
                                firstA = False
                                nc.tensor.matmul(
                                    ps_ctxB[:, c0 + P:], vs[jk],
                                    pr2[:, SQ + c0 + P:],
                                    start=firstB, stop=False)
                                firstB = False
                            nc.tensor.matmul(
                                ps_ctxA[:, c0:c0 + P], vs[jk], prm[:, :P],
                                start=firstA, stop=last)
                            firstA = False
                            nc.tensor.matmul(
                                ps_ctxB[:, c0:c0 + P], vs[jk], prm[:, P:],
                                start=firstB, stop=last)
                            firstB = False
                        else:
                            nc.tensor.matmul(ps_ctxA, vs[jk], pr2[:, :SQ],
                                             start=firstA, stop=False)
                            firstA = False
                            nc.tensor.matmul(ps_ctxB, vs[jk], pr2[:, SQ:],
                                             start=firstB, stop=False)
                            firstB = False
                    stashes = [stash_ctx(2 * pr, q, ps_ctxA),
                               stash_ctx(2 * pr + 1, q, ps_ctxB)]
                    for d in pend_den:
                        emit_den(*d)
                    pend_den = stashes
                    # out-proj filler keeps the PE saturated while the
                    # ACT engine grinds through the exps
                    for _ in range(4):
                        if pend_chunks:
                            op_chunk(*pend_chunks.pop(0))
            for d in pend_den:
                emit_den(*d)
            for ch in pend_chunks:
                op_chunk(*ch)
            for jj in range(4):
                for n2 in range(2):
                    op_chunk(NSQ - 1, jj, n2)

    nc.compile()
    return nc


def prep_inputs(x, cos, sin, Wq, Wk, Wv, Wo, q_norm_w, k_norm_w):
    """Host-side shard + layout prep. Returns per-core input maps."""
    xtn = np.ascontiguousarray(x.reshape(S, D).T).astype(NBF)

    # rope tables with (1 + norm_w) folded in, k-block appended, and the
    # sin first-half pre-negated (so rope is out = q*cos5 + rot(q)*sin5
    # with rot(q) = [q2, q1])
    half = HD // 2
    wq1 = (1.0 + q_norm_w).astype(np.float32)
    wk1 = (1.0 + k_norm_w).astype(np.float32)

    def rotw(w):
        return np.concatenate([w[half:], w[:half]])

    sin_m = sin.copy()
    sin_m[:, :half] = -sin_m[:, :half]
    cos_q = cos * wq1
    cos_k = cos * wk1
    sin_q = sin_m * rotw(wq1)
    sin_k = sin_m * rotw(wk1)
    cos5 = np.concatenate([np.tile(cos_q, (1, NHL)), cos_k], axis=1)
    sin5 = np.concatenate([np.tile(sin_q, (1, NHL)), sin_k], axis=1)
    cos5 = np.ascontiguousarray(cos5).astype(NBF)
    sin5 = np.ascontiguousarray(sin5).astype(NBF)

    tri = np.triu(np.ones((P, P), dtype=np.float32)).astype(NBF)  # p<=f

    Wqh = Wq.reshape(H, 2 * HD, D)
    in_maps = []
    for c in range(NCORE):
        hs = slice(NHL * c, NHL * (c + 1))
        wq_c = Wqh[hs, :HD, :].reshape(EL, D)       # q rows, 4 heads
        wgt_c = Wqh[hs, HD:, :].reshape(EL, D)      # gate rows
        wk_c = Wk[HD * c:HD * (c + 1), :]
        wv_c = Wv[HD * c:HD * (c + 1), :]
        wqkv_c = np.concatenate([wq_c, wk_c, wv_c], axis=0)  # [384, D]
        in_maps.append({
            "xt": xtn,
            "wqkv": np.ascontiguousarray(wqkv_c.T).astype(NBF),
            "wg": np.ascontiguousarray(wgt_c.T).astype(NBF),
            "wo": np.ascontiguousarray(
                Wo[:, EL * c:EL * (c + 1)].T).astype(NBF),
            "cos5": cos5,
            "sin5": sin5,
            "tri": tri,
        })
    return in_maps


_NC_CACHE = {}


def get_nc():
    if "nc" not in _NC_CACHE:
        _NC_CACHE["nc"] = build_nc()
    return _NC_CACHE["nc"]


def run(in_maps, trace=False, **kw):
    nc = get_nc()
    return run_bass_kernel_spmd(nc, in_maps, list(range(NCORE)),
                                trace=trace, **kw)


def kernel(x, mask, cos, sin, Wq, Wk, Wv, Wo, q_norm_w, k_norm_w):
    in_maps = prep_inputs(np.asarray(x, dtype=np.float32), np.asarray(cos),
                          np.asarray(sin), np.asarray(Wq), np.asarray(Wk),
                          np.asarray(Wv), np.asarray(Wo),
                          np.asarray(q_norm_w), np.asarray(k_norm_w))
    res = run(in_maps)
    acc = np.zeros((S, D), dtype=np.float32)
    for r in res.results:
        acc += np.asarray(r["out"], dtype=np.float32)
    return acc.reshape(1, S, D)
